# revision 8
# baseline (speedup 1.0000x reference)
"""Mueller-matrix pyramid kernel for Trainium2 (8 NeuronCores).

Sharding: 8 cores = (batch 4) x (H-halves 2). Each core computes the full
51-channel output for its 256-row half at 512 cols.

Per-core layout: channel-planes with pixels on [partitions=rows, free=cols].
- Mueller features: DVE tensor_tensor chain (adjugate/cofactor form; the
  det(A)*det(W) scale cancels in the m00 normalization), ACT reciprocal.
- Pyramid pooling: W-pool = strided DVE max; H-pool = partition-strided
  SBUF->SBUF DMA gathers + DVE max.
- Levels 1/2 features share one DVE pass on packed [68, 136] planes
  (lvl1 cols 0:128, lvl2 packed into cols 128:136; 68*8 == 17*32).
- Bilinear upsample (align_corners): PE matmuls (transpose -> W-matmul with
  the data as stationary operand -> H-matmul with per-core R matrices).

SPMD uniformity: halo rows (16 extra pooling rows) arrive in a separate
input tensor; their pooled rows live at fixed local positions and the
per-core R1/R2 matrices absorb the local->global row permutation, so one
program serves both halves.
"""

import numpy as np

H = W = 512
CIN = 48
LEVELS = 3
HALF = 256          # output rows per core
HALO = 16           # extra pooling rows per core
L1R = 68            # local level-1 rows (64 main + 4 halo)
L1W = 128
L2R = 17            # local level-2 rows (16 main + 1 halo)
L2W = 32
PACK2 = 8           # lvl2 packed cols per partition-row (68*8 = 17*32)
FTW = L1W + PACK2   # 136
N_CORES = 8
CW = 256            # level-0 col-tile width

# ---------------------------------------------------------------------------
# host-side constants
# ---------------------------------------------------------------------------


def _interp_1d(n_out, n_in, lo, hi):
    out = np.zeros((hi - lo, n_in), np.float32)
    scale = (n_in - 1.0) / (n_out - 1.0)
    for i, y in enumerate(range(lo, hi)):
        t = np.float32(y * scale)
        y0 = int(np.floor(t))
        fy = np.float32(t - y0)
        y1 = min(y0 + 1, n_in - 1)
        out[i, y0] += np.float32(1.0) - fy
        out[i, y1] += fy
    return out


def _r_matrix(half, n_in, n_main, off_main, off_halo, n_halo):
    lo, hi = half * HALF, half * HALF + HALF
    full = _interp_1d(H, n_in, lo, hi)
    loc = np.zeros((HALF, n_main + n_halo), np.float32)
    loc[:, :n_main] = full[:, off_main:off_main + n_main]
    loc[:, n_main:] = full[:, off_halo:off_halo + n_halo]
    return loc


def _host_constants(half):
    if half == 0:
        r1 = _r_matrix(0, 128, 64, 0, 64, 4)
        r2 = _r_matrix(0, 32, 16, 0, 16, 1)
    else:
        r1 = _r_matrix(1, 128, 64, 64, 60, 4)
        r2 = _r_matrix(1, 32, 16, 16, 15, 1)
    c1 = _interp_1d(W, L1W, 0, W).T.copy()
    c2 = _interp_1d(W, L2W, 0, W).T.copy()
    return (np.ascontiguousarray(r1.T), np.ascontiguousarray(r2.T),
            np.ascontiguousarray(c1), np.ascontiguousarray(c2))


# ---------------------------------------------------------------------------
# op tables (adjugate via cofactors); a[k] = entry (k//4, k%4)
# ---------------------------------------------------------------------------

# minors m = a[e1]*a[e2] - a[e3]*a[e4]
# S-minors s0..s5 (rows 0,1), C-minors c0..c5 (rows 2,3)
_SMIN = [
    (0, 5, 4, 1), (0, 6, 4, 2), (0, 7, 4, 3),
    (1, 6, 5, 2), (1, 7, 5, 3), (2, 7, 6, 3),
]
_CMIN = [
    (8, 13, 12, 9), (8, 14, 12, 10), (8, 15, 12, 11),
    (9, 14, 13, 10), (9, 15, 13, 11), (10, 15, 14, 11),
]
# adj[flat] = sign*(a[x1]*m1 - a[x2]*m2 + a[x3]*m3); minors: ('c'|'s', idx)
_ADJ = {
    0:  (+1, (5, 'c', 5), (6, 'c', 4), (7, 'c', 3)),
    1:  (-1, (1, 'c', 5), (2, 'c', 4), (3, 'c', 3)),
    2:  (+1, (13, 's', 5), (14, 's', 4), (15, 's', 3)),
    3:  (-1, (9, 's', 5), (10, 's', 4), (11, 's', 3)),
    4:  (-1, (4, 'c', 5), (6, 'c', 2), (7, 'c', 1)),
    5:  (+1, (0, 'c', 5), (2, 'c', 2), (3, 'c', 1)),
    6:  (-1, (12, 's', 5), (14, 's', 2), (15, 's', 1)),
    7:  (+1, (8, 's', 5), (10, 's', 2), (11, 's', 1)),
    8:  (+1, (4, 'c', 4), (5, 'c', 2), (7, 'c', 0)),
    9:  (-1, (0, 'c', 4), (1, 'c', 2), (3, 'c', 0)),
    10: (+1, (12, 's', 4), (13, 's', 2), (15, 's', 0)),
    11: (-1, (8, 's', 4), (9, 's', 2), (11, 's', 0)),
    12: (-1, (4, 'c', 3), (5, 'c', 1), (6, 'c', 0)),
    13: (+1, (0, 'c', 3), (1, 'c', 1), (2, 'c', 0)),
    14: (-1, (12, 's', 3), (13, 's', 1), (14, 's', 0)),
    15: (+1, (8, 's', 3), (9, 's', 1), (10, 's', 0)),
}

_NC_CACHE = {}


def _build_nc():
    import concourse.bacc as bacc
    import concourse.mybir as mybir
    from concourse.tile import TileContext
    from concourse.masks import make_identity

    f32 = mybir.dt.float32
    ALU = mybir.AluOpType
    AF = mybir.ActivationFunctionType

    nc = bacc.Bacc("TRN2", target_bir_lowering=False, num_devices=N_CORES)

    xmm = nc.dram_tensor("xmm", [CIN, HALF, W], f32, kind="ExternalInput")
    xhalo = nc.dram_tensor("xhalo", [CIN, HALO, W], f32, kind="ExternalInput")
    r1t = nc.dram_tensor("r1t", [L1R, HALF], f32, kind="ExternalInput")
    r2t = nc.dram_tensor("r2t", [L2R, HALF], f32, kind="ExternalInput")
    c1 = nc.dram_tensor("c1", [L1W, W], f32, kind="ExternalInput")
    c2 = nc.dram_tensor("c2", [L2W, W], f32, kind="ExternalInput")
    out = nc.dram_tensor("out", [17 * LEVELS, HALF, W], f32, kind="ExternalOutput")
    import os
    dbg_en = os.environ.get("KDBG") == "1"
    dbg = (nc.dram_tensor("dbg", [L1R, CIN, FTW], f32, kind="ExternalOutput")
           if dbg_en else None)

    def TT(o, a, b, op):
        nc.vector.tensor_tensor(out=o, in0=a, in1=b, op=op)

    def mueller(pool_t, FD, xI, xA, xW, opl, rs):
        """Emit the 48->17 Mueller feature chain on [rs, FD] planes.
        xI/xA/xW: accessor(e)->plane AP for matrix entry e (flat 0..15).
        opl(k): output plane (0 = intensity, 1+4i+j = M[i,j])."""
        mnr = pool_t.tile([128, 6, FD], f32, tag="mnr")
        adjc = pool_t.tile([128, 4, FD], f32, tag="adjc")
        pp = pool_t.tile([128, 16, FD], f32, tag="pp")
        ncol = pool_t.tile([128, 4, FD], f32, tag="ncol")
        scr = pool_t.tile([128, 2, FD], f32, tag="scr")

        def pl(t, k):
            return t[0:rs, k]

        s0, s1 = pl(scr, 0), pl(scr, 1)

        # intensity
        TT(s1, xI(0), xI(1), ALU.add)
        for k in range(2, 16):
            TT(s1, s1, xI(k), ALU.add)
        nc.scalar.mul(opl(0), s1, 1.0 / 16.0)

        def emit_minors(xE, table):
            for i, (e1, e2, e3, e4) in enumerate(table):
                TT(pl(mnr, i), xE(e1), xE(e2), ALU.mult)
                TT(s0, xE(e3), xE(e4), ALU.mult)
                TT(pl(mnr, i), pl(mnr, i), s0, ALU.subtract)

        def emit_adj_entry(xE, dst, flat):
            sgn, t1, t2, t3 = _ADJ[flat]
            def mslot(t):
                return pl(mnr, t[2])
            if sgn > 0:
                TT(dst, xE(t1[0]), mslot(t1), ALU.mult)
                TT(s0, xE(t2[0]), mslot(t2), ALU.mult)
                TT(dst, dst, s0, ALU.subtract)
                TT(s0, xE(t3[0]), mslot(t3), ALU.mult)
                TT(dst, dst, s0, ALU.add)
            else:
                TT(dst, xE(t2[0]), mslot(t2), ALU.mult)
                TT(s0, xE(t1[0]), mslot(t1), ALU.mult)
                TT(dst, dst, s0, ALU.subtract)
                TT(s0, xE(t3[0]), mslot(t3), ALU.mult)
                TT(dst, dst, s0, ALU.subtract)

        # ---- P = adj(A) @ I, accumulated column-by-column of adjA ----
        for mtype, table, cols in (('c', _CMIN, (0, 1)), ('s', _SMIN, (2, 3))):
            emit_minors(xA, table)
            for k in cols:
                for i in range(4):
                    emit_adj_entry(xA, pl(adjc, i), 4 * i + k)
                for i in range(4):
                    for j in range(4):
                        o = 4 * i + j
                        if k == 0:
                            TT(pl(pp, o), pl(adjc, i), xI(4 * 0 + j), ALU.mult)
                        else:
                            TT(s0, pl(adjc, i), xI(4 * k + j), ALU.mult)
                            TT(pl(pp, o), pl(pp, o), s0, ALU.add)

        # ---- N = P @ adj(W), column-by-column; M = N / N00 ----
        rec = s1
        for mtype, table, cols in (('c', _CMIN, (0, 1)), ('s', _SMIN, (2, 3))):
            emit_minors(xW, table)
            for j in cols:
                for k in range(4):
                    emit_adj_entry(xW, pl(adjc, k), 4 * k + j)
                for i in range(4):
                    TT(pl(ncol, i), pl(pp, 4 * i), pl(adjc, 0), ALU.mult)
                    for k in range(1, 4):
                        TT(s0, pl(pp, 4 * i + k), pl(adjc, k), ALU.mult)
                        TT(pl(ncol, i), pl(ncol, i), s0, ALU.add)
                if j == 0:
                    nc.vector.reciprocal(rec, pl(ncol, 0))
                for i in range(4):
                    TT(opl(1 + 4 * i + j), pl(ncol, i), rec, ALU.mult)

    with TileContext(nc) as tc:
        with (
            tc.tile_pool(name="cst", bufs=1) as pool_c,
            tc.tile_pool(name="lvl", bufs=1) as pool_l,
        ):
            ident = pool_c.tile([128, 128], f32)
            make_identity(nc, ident)
            r1t_s = pool_c.tile([L1R, HALF], f32)
            nc.sync.dma_start(out=r1t_s, in_=r1t[:, :])
            r2t_s = pool_c.tile([L2R, HALF], f32)
            nc.sync.dma_start(out=r2t_s, in_=r2t[:, :])
            c1_s = pool_c.tile([L1W, W], f32)
            nc.sync.dma_start(out=c1_s, in_=c1[:, :])
            c2_s = pool_c.tile([L2W, W], f32)
            nc.sync.dma_start(out=c2_s, in_=c2[:, :])

            # persistent level-1(+packed lvl2) planes
            lvl1 = pool_l.tile([L1R, CIN, FTW], f32)

            # ================= phase 0: level-0 tiles + pooling ============
            with (
                tc.tile_pool(name="px", bufs=1) as pool_x,
                tc.tile_pool(name="pt", bufs=1) as pool_t,
                tc.tile_pool(name="po", bufs=1) as pool_o,
                tc.tile_pool(name="pp", bufs=1) as pool_p,
                tc.tile_pool(name="phl", bufs=1) as pool_h,
            ):
                for rt in range(2):
                    for ct in range(2):
                        r0, c0 = rt * 128, ct * CW
                        chunks = []
                        for ci, tg in ((0, "xi"), (1, "xa"), (2, "xw")):
                            t = pool_x.tile([128, 16, CW], f32, tag=tg)
                            nc.sync.dma_start(
                                out=t,
                                in_=xmm[16 * ci:16 * ci + 16, r0:r0 + 128,
                                        c0:c0 + CW].rearrange("c r w -> r c w"))
                            chunks.append(t)
                        xi, xa, xw = chunks
                        ot = pool_o.tile([128, 17, CW], f32, tag="ot")
                        mueller(pool_t, CW,
                                lambda e: xi[:, e], lambda e: xa[:, e],
                                lambda e: xw[:, e], lambda k: ot[:, k], 128)
                        nc.sync.dma_start(
                            out=out[0:17, r0:r0 + 128, c0:c0 + CW]
                            .rearrange("c r w -> r c w"),
                            in_=ot)
                        # W-pool all 48 chans -> wp [128, 48, 64]
                        wp = pool_p.tile([128, CIN, CW // 4], f32, tag="wp")
                        for ci, t in enumerate(chunks):
                            t4 = t.rearrange("r c (w f) -> r c w f", f=4)
                            wps = wp[:, 16 * ci:16 * ci + 16]
                            TT(wps, t4[:, :, :, 0], t4[:, :, :, 1], ALU.max)
                            TT(wps, wps, t4[:, :, :, 2], ALU.max)
                            TT(wps, wps, t4[:, :, :, 3], ALU.max)
                        # H-pool -> lvl1[32rt:32rt+32, :, 64ct:64ct+64]
                        wpr = wp.rearrange("(r f) c w -> r f c w", f=4)
                        dst = lvl1[32 * rt:32 * rt + 32, :,
                                   64 * ct:64 * ct + 64]
                        nc.sync.dma_start(out=dst, in_=wpr[:, 0])
                        gt = pool_p.tile([128, CIN, CW // 4], f32, tag="gt")
                        gts = gt[32 * rt:32 * rt + 32, :, 0:64]
                        for dy in (1, 2, 3):
                            nc.sync.dma_start(out=gts, in_=wpr[:, dy])
                            TT(dst, dst, gts, ALU.max)

                # halo strip, rows-on-partitions like the main path
                HCW = 128
                for ct in range(W // HCW):
                    c0 = ct * HCW
                    xh = pool_h.tile([HALO, CIN, HCW], f32, tag="xh")
                    nc.sync.dma_start(
                        out=xh,
                        in_=xhalo[:, :, c0:c0 + HCW].rearrange("c r w -> r c w"))
                    wph = pool_h.tile([HALO, CIN, HCW // 4], f32, tag="wph")
                    xh4 = xh.rearrange("r c (w f) -> r c w f", f=4)
                    TT(wph, xh4[:, :, :, 0], xh4[:, :, :, 1], ALU.max)
                    TT(wph, wph, xh4[:, :, :, 2], ALU.max)
                    TT(wph, wph, xh4[:, :, :, 3], ALU.max)
                    wphr = wph.rearrange("(r f) c w -> r f c w", f=4)
                    dst = lvl1[64:68, :, 32 * ct:32 * ct + 32]
                    nc.sync.dma_start(out=dst, in_=wphr[:, 0])
                    gh = pool_h.tile([68, CIN, HCW // 4], f32, tag="gh")
                    ghs = gh[64:68]
                    for dy in (1, 2, 3):
                        nc.sync.dma_start(out=ghs, in_=wphr[:, dy])
                        TT(dst, dst, ghs, ALU.max)

            # ================= phase 1: levels 1+2 ========================
            with (
                tc.tile_pool(name="pt2", bufs=1) as pool_t2,
                tc.tile_pool(name="pl2", bufs=1) as pool_2,
                tc.tile_pool(name="po2", bufs=1) as pool_o2,
                tc.tile_pool(name="pu", bufs=2) as pool_u,
                tc.tile_pool(name="psu", bufs=2, space="PSUM") as pool_ps,
            ):
                # --- level-2 build into lvl1 cols 128:136 ---
                wp2 = pool_2.tile([L1R, CIN, L2W], f32, tag="wp2")
                l14 = lvl1[:, :, 0:L1W].rearrange("r c (w f) -> r c w f", f=4)
                TT(wp2, l14[:, :, :, 0], l14[:, :, :, 1], ALU.max)
                TT(wp2, wp2, l14[:, :, :, 2], ALU.max)
                TT(wp2, wp2, l14[:, :, :, 3], ALU.max)
                wp2r = wp2[0:64].rearrange("(r f) c w -> r f c w", f=4)
                l2 = pool_2.tile([L2R, CIN, L2W], f32, tag="l2")
                l2m = l2[0:16]
                nc.sync.dma_start(out=l2m, in_=wp2r[:, 0])
                g2 = pool_2.tile([L2R, CIN, L2W], f32, tag="g2")
                g2m = g2[0:16]
                for dy in (1, 2, 3):
                    nc.sync.dma_start(out=g2m, in_=wp2r[:, dy])
                    TT(l2m, l2m, g2m, ALU.max)
                # halo row staged at partition 0 (engine partition bases must
                # be 32-aligned), then DMA'd into l2 row 16
                th = pool_2.tile([1, CIN, L2W], f32, tag="th")
                gh2 = pool_2.tile([1, CIN, L2W], f32, tag="gh2")
                nc.sync.dma_start(out=th, in_=wp2[64:65])
                for dy in (1, 2, 3):
                    nc.sync.dma_start(out=gh2, in_=wp2[64 + dy:65 + dy])
                    TT(th, th, gh2, ALU.max)
                nc.sync.dma_start(out=l2[16:17], in_=th)
                lvl1p = lvl1.rearrange("(r f) c w -> r f c w", f=4)
                for m in range(4):
                    nc.sync.dma_start(out=lvl1p[0:17, m, :, L1W:FTW],
                                      in_=l2[:, :, 8 * m:8 * m + 8])

                if dbg is not None:
                    nc.sync.dma_start(out=dbg[:, :, :], in_=lvl1)

                # --- level-1+2 features on packed planes ---
                ot2 = pool_o2.tile([L1R, 17, FTW], f32, tag="ot2")
                mueller(pool_t2, FTW,
                        lambda e: lvl1[0:L1R, e],
                        lambda e: lvl1[0:L1R, 16 + e],
                        lambda e: lvl1[0:L1R, 32 + e],
                        lambda k: ot2[0:L1R, k], L1R)

                # unpack lvl2 features -> l2f [17, 17, 32]
                l2f = pool_o2.tile([L2R, 17, L2W], f32, tag="l2f")
                ot2p = ot2.rearrange("(r f) k w -> r f k w", f=4)
                for m in range(4):
                    nc.sync.dma_start(out=l2f[:, :, 8 * m:8 * m + 8],
                                      in_=ot2p[0:17, m, :, L1W:FTW])

                # --- upsample via PE ---
                def upsample(nch_base, rloc, wloc, rts, cs, plane_fn):
                    for chn in range(17):
                        plane = plane_fn(chn)
                        pst = pool_ps.tile([128, 128], f32, tag="pst")
                        nc.tensor.transpose(pst[0:wloc, 0:rloc], plane,
                                            ident[0:rloc, 0:rloc])
                        pts = pool_u.tile([L1W, L1R], f32, tag="pts")
                        nc.scalar.copy(pts[0:wloc, 0:rloc],
                                       pst[0:wloc, 0:rloc])
                        psw = pool_ps.tile([L1R, W], f32, tag="psw")
                        nc.tensor.matmul(psw[0:rloc, :], pts[0:wloc, 0:rloc],
                                         cs[0:wloc, :], start=True, stop=True)
                        wres = pool_u.tile([L1R, W], f32, tag="wres")
                        nc.scalar.copy(wres[0:rloc, :], psw[0:rloc, :])
                        for yb in range(2):
                            psf = pool_ps.tile([128, W], f32, tag="psf")
                            nc.tensor.matmul(
                                psf, rts[0:rloc, yb * 128:(yb + 1) * 128],
                                wres[0:rloc, :], start=True, stop=True)
                            fin = pool_u.tile([128, W], f32, tag="fin")
                            if chn % 2 == 0:
                                nc.scalar.copy(fin, psf)
                            else:
                                nc.vector.tensor_copy(out=fin, in_=psf)
                            nc.sync.dma_start(
                                out=out[nch_base + chn,
                                        yb * 128:(yb + 1) * 128, :],
                                in_=fin)

                upsample(17, L1R, L1W, r1t_s, c1_s,
                         lambda chn: ot2[0:L1R, chn, 0:L1W])
                upsample(34, L2R, L2W, r2t_s, c2_s,
                         lambda chn: l2f[:, chn, :])

    nc.compile()
    return nc


def kernel(x: np.ndarray) -> np.ndarray:
    from concourse.bass_utils import run_bass_kernel_spmd

    x = np.ascontiguousarray(x, dtype=np.float32)
    B = x.shape[0]
    assert x.shape == (4, CIN, H, W), x.shape

    if "nc" not in _NC_CACHE:
        _NC_CACHE["nc"] = _build_nc()
    nc = _NC_CACHE["nc"]

    consts = [_host_constants(0), _host_constants(1)]
    in_maps = []
    for core in range(N_CORES):
        b, half = core // 2, core % 2
        r1tc, r2tc, c1c, c2c = consts[half]
        if half == 0:
            xm = x[b, :, 0:HALF, :]
            xh = x[b, :, HALF:HALF + HALO, :]
        else:
            xm = x[b, :, HALF:2 * HALF, :]
            xh = x[b, :, HALF - HALO:HALF, :]
        in_maps.append({
            "xmm": np.ascontiguousarray(xm),
            "xhalo": np.ascontiguousarray(xh),
            "r1t": r1tc, "r2t": r2tc, "c1": c1c, "c2": c2c,
        })

    res = run_bass_kernel_spmd(nc, in_maps, core_ids=list(range(N_CORES)))
    outv = np.empty((B, 17 * LEVELS, H, W), np.float32)
    for core in range(N_CORES):
        b, half = core // 2, core % 2
        outv[b, :, half * HALF:(half + 1) * HALF, :] = res.results[core]["out"]
    return outv
